# revision 1
# baseline (speedup 1.0000x reference)
"""CollectAtomTriples Trainium2 kernel.

Input: idx_i -- sorted int32 center indices [N_PAIRS] forming ragged segments.
Output: (idx_i_triples, idx_j_triples, idx_k_triples) -- for every segment of
length c, all C(c,2) unordered neighbor pairs (a<b, lexicographic), emitting
(segment_id, seg_start+a, seg_start+b) at data-dependent total length T.

Strategy (v3): host finds segment boundaries and splits segments contiguously
across 8 cores balanced by triple count.  Segments are grouped by count-class
c; all segments of one class share local patterns pat_a/pat_b =
np.triu_indices(c,1), so each output row is base[s] + pattern -- a
per-partition broadcast add.  Layout is column-blocked: class c gets
ceil(H_c/128) column blocks of width M=C(c,2); segment q*128+p of the class
lives at partition p, column block q.  Blocks are greedy-packed into [128, F]
tiles; each tile is ONE big HWDGE dma_start (~1.5MB, 12KB descriptors) into a
per-tile scratch rectangle -- no SWDGE descriptor generation (v1 bottleneck)
and only ~60 DMA issues total (v2 bottleneck was ~770 small issues +
serialized per-class PE broadcast chains).  Patterns are broadcast to 128
partitions in bulk (one SBUF->SBUF SWDGE DMA per phase of classes).  The
host applies the static scratch->output permutation during gather/unshard.
Add streams alternate DVE/ACT to stay under the HBM write roofline.
"""

import numpy as np

N_CORES = 8
P = 128
F_MAX = 3072  # tile free-dim elems (12KB int32 per partition)
PHASE_M = 3072  # max sum of class pattern widths per phase


def _plan(idx, n_cores):
    idx = np.asarray(idx)
    n = idx.shape[0]
    starts = np.concatenate(
        [[0], np.flatnonzero(idx[1:] != idx[:-1]) + 1]
    ).astype(np.int64)
    counts = np.diff(np.concatenate([starts, [n]]))
    tri_counts = counts * (counts - 1) // 2
    ctri = np.cumsum(tri_counts)
    T = int(ctri[-1])
    tri_off = ctri - tri_counts  # exclusive scan
    seg_off = starts

    sel = np.flatnonzero(tri_counts > 0)  # segments with c >= 2
    sc = counts[sel].astype(np.int64)
    soff = seg_off[sel]
    stri = tri_off[sel]
    stric = tri_counts[sel]

    # contiguous split of segments across cores, balanced by triple count
    csum = np.cumsum(stric)
    cuts = [0]
    for k in range(1, n_cores):
        cuts.append(int(np.searchsorted(csum, (T * k) // n_cores, side="left")))
    cuts.append(sel.size)
    cuts = sorted(cuts)

    # count classes and per-core class histograms
    classes = np.unique(sc)
    n_classes = classes.size
    n_ck = np.zeros((n_cores, n_classes), np.int64)
    core_cidx = []
    for k in range(n_cores):
        cidx = np.searchsorted(classes, sc[cuts[k]:cuts[k + 1]])
        core_cidx.append(cidx)
        n_ck[k] = np.bincount(cidx, minlength=n_classes)
    H = n_ck.max(axis=0)

    # patterns (lexicographic (a,b), a<b), int32 flat tables
    M_of = np.array([int(c) * (int(c) - 1) // 2 for c in classes])
    pa_chunks, pb_chunks = [], []
    for c in classes:
        a, b = np.triu_indices(int(c), 1)
        pa_chunks.append(a.astype(np.int32))
        pb_chunks.append(b.astype(np.int32))
    pat_a = np.concatenate(pa_chunks)[None, :]
    pat_b = np.concatenate(pb_chunks)[None, :]
    pat_table_off = np.concatenate([[0], np.cumsum(M_of)[:-1]])
    L = int(M_of.sum())

    # phases: consecutive classes with sum(M) <= PHASE_M
    phases = []
    cur, cur_m = [], 0
    for ci in range(n_classes):
        if cur and cur_m + M_of[ci] > PHASE_M:
            phases.append(cur)
            cur, cur_m = [], 0
        cur.append(ci)
        cur_m += int(M_of[ci])
    if cur:
        phases.append(cur)

    # column blocks (ci, q); greedy-packed into [128, F<=F_MAX] tiles
    blocks = []  # meta column index == position in this list
    block_col = {}
    phase_info = []  # (pat_off0, Lp, tiles); tile = (scratch_off, F, blocklist)
    scratch_off = 0
    for phase in phases:
        p0 = int(pat_table_off[phase[0]])
        Lp = int(sum(M_of[ci] for ci in phase))
        tiles = []
        tb, tw = [], 0
        for ci in phase:
            M = int(M_of[ci])
            ncols = max(1, -(-int(H[ci]) // P))
            for q in range(ncols):
                if tw + M > F_MAX and tb:
                    tiles.append((scratch_off, tw, tb))
                    scratch_off += P * tw
                    tb, tw = [], 0
                b = len(blocks)
                blocks.append((ci, q))
                block_col[(ci, q)] = b
                tb.append((ci, q, tw, int(pat_table_off[ci]) - p0, M, b))
                tw += M
        if tb:
            tiles.append((scratch_off, tw, tb))
            scratch_off += P * tw
        phase_info.append((p0, Lp, tiles))
    B = len(blocks)
    S_total = scratch_off

    # slot address: (ci, q) -> (tile scratch offset, tile F, col0)
    slot_addr = {}
    for _, _, tiles in phase_info:
        for toff, F, tb in tiles:
            for ci, q, col0, _, M, b in tb:
                slot_addr[(ci, q)] = (toff, F, col0)

    # per-core metadata [P, B] + host-side gather permutation
    meta_segid = np.zeros((n_cores, P, B), np.int32)
    meta_base = np.zeros((n_cores, P, B), np.int32)
    perm = np.empty(T, np.int64)
    for k in range(n_cores):
        s0 = cuts[k]
        cidx = core_cidx[k]
        order = np.argsort(cidx, kind="stable")
        pos = 0
        core_base = k * S_total
        for ci in range(n_classes):
            cnt = int(n_ck[k, ci])
            if cnt == 0:
                continue
            gsel = s0 + order[pos:pos + cnt]  # ascending segment order
            pos += cnt
            M = int(M_of[ci])
            nn = np.arange(cnt)
            qs, ps = nn // P, nn % P
            cols = np.array([block_col[(ci, int(q))] for q in qs])
            meta_segid[k, ps, cols] = sel[gsel].astype(np.int32)
            meta_base[k, ps, cols] = soff[gsel].astype(np.int32)
            addr = np.empty(cnt, np.int64)
            for q in np.unique(qs):
                toff, F, col0 = slot_addr[(ci, int(q))]
                m = qs == q
                addr[m] = toff + ps[m] * F + col0
            src = core_base + addr
            dst = stri[gsel]
            perm_idx = (dst[:, None] + np.arange(M)[None, :]).ravel()
            perm_val = (src[:, None] + np.arange(M)[None, :]).ravel()
            perm[perm_idx] = perm_val

    in_maps = [
        {
            "meta_segid": meta_segid[k],
            "meta_base": meta_base[k],
            "meta_segid_f": meta_segid[k].astype(np.float32),
            "meta_base_f": meta_base[k].astype(np.float32),
            "pat_a": pat_a,
            "pat_b": pat_b,
        }
        for k in range(n_cores)
    ]
    return {
        "B": B,
        "phase_info": phase_info,
        "M_max": int(M_of.max()),
        "Lp_max": max(Lp for _, Lp, _ in phase_info),
        "pat_len": L,
        "T": T,
        "S_total": S_total,
        "perm": perm,
        "in_maps": in_maps,
        "n_cores": n_cores,
    }


def _build_program(plan):
    import concourse.bacc as bacc
    import concourse.bass as bass
    import concourse.mybir as mybir
    import concourse.tile as tile

    B = plan["B"]
    L = plan["pat_len"]
    S_total = plan["S_total"]
    M_max = plan["M_max"]
    Lp_max = plan["Lp_max"]
    i32 = mybir.dt.int32
    f32 = mybir.dt.float32

    nc = bacc.Bacc(
        "TRN2",
        target_bir_lowering=False,
        debug=False,
        num_devices=plan["n_cores"],
    )
    m_segid_d = nc.dram_tensor("meta_segid", [P, B], i32, kind="ExternalInput")
    m_base_d = nc.dram_tensor("meta_base", [P, B], i32, kind="ExternalInput")
    m_segid_f_d = nc.dram_tensor("meta_segid_f", [P, B], f32, kind="ExternalInput")
    m_base_f_d = nc.dram_tensor("meta_base_f", [P, B], f32, kind="ExternalInput")
    pat_a_d = nc.dram_tensor("pat_a", [1, L], i32, kind="ExternalInput")
    pat_b_d = nc.dram_tensor("pat_b", [1, L], i32, kind="ExternalInput")
    out_d = {
        name: nc.dram_tensor(name, [S_total, 1], i32, kind="ExternalOutput")
        for name in ("out_i", "out_j", "out_k")
    }

    alt = 0
    with tile.TileContext(nc) as tc:
        with (
            tc.tile_pool(name="meta", bufs=1) as meta_pool,
            tc.tile_pool(name="const", bufs=1) as const_pool,
            tc.tile_pool(name="patrow", bufs=2) as patrow_pool,
            tc.tile_pool(name="pat", bufs=2) as pat_pool,
            tc.tile_pool(name="work", bufs=2) as work_pool,
        ):
            m_segid = meta_pool.tile([P, B], i32, tag="msegid")
            m_base = meta_pool.tile([P, B], i32, tag="mbase")
            m_segid_f = meta_pool.tile([P, B], f32, tag="msegidf")
            m_base_f = meta_pool.tile([P, B], f32, tag="mbasef")
            nc.sync.dma_start(out=m_segid[:], in_=m_segid_d.ap())
            nc.sync.dma_start(out=m_base[:], in_=m_base_d.ap())
            nc.sync.dma_start(out=m_segid_f[:], in_=m_segid_f_d.ap())
            nc.sync.dma_start(out=m_base_f[:], in_=m_base_f_d.ap())

            zeros = const_pool.tile([P, M_max], i32, tag="zeros")
            nc.vector.memset(zeros[:], 0)

            for p0, Lp, tiles in plan["phase_info"]:
                pa = pat_pool.tile([P, Lp_max], i32, tag="pa")
                pb = pat_pool.tile([P, Lp_max], i32, tag="pb")
                # replicate pattern row to all partitions: DRAM broadcast to
                # 32 partitions (step-0 partition AP is legal for DRAM src),
                # then two wide SBUF->SBUF hops 32->64->128 (depth 3, vs the
                # 8-deep serial doubling tree that dominated the v3 span)
                for src_d, dst in ((pat_a_d, pa), (pat_b_d, pb)):
                    nc.gpsimd.dma_start(
                        out=dst[0:32, :Lp],
                        in_=bass.AP(
                            tensor=src_d, offset=p0, ap=[[0, 32], [1, Lp]]
                        ),
                    )
                    nc.gpsimd.dma_start(
                        out=dst[32:64, :Lp], in_=dst[0:32, :Lp]
                    )
                    nc.gpsimd.dma_start(
                        out=dst[64:128, :Lp], in_=dst[0:64, :Lp]
                    )

                for toff, F, tb in tiles:
                    ti = work_pool.tile([P, F_MAX], i32, tag="ti")
                    tj = work_pool.tile([P, F_MAX], i32, tag="tj")
                    tk = work_pool.tile([P, F_MAX], i32, tag="tk")
                    for ci, q, col0, poff, M, b in tb:
                        sl = slice(col0, col0 + M)
                        psl = slice(poff, poff + M)
                        nc.scalar.activation(
                            out=ti[:, sl],
                            in_=zeros[:, :M],
                            func=mybir.ActivationFunctionType.Identity,
                            bias=m_segid_f[:, b:b + 1],
                        )
                        nc.vector.tensor_tensor(
                            out=tj[:, sl],
                            in0=pa[:, psl],
                            in1=m_base[:, b:b + 1].to_broadcast([P, M]),
                            op=mybir.AluOpType.add,
                        )
                        if alt == 0:
                            nc.vector.tensor_tensor(
                                out=tk[:, sl],
                                in0=pb[:, psl],
                                in1=m_base[:, b:b + 1].to_broadcast([P, M]),
                                op=mybir.AluOpType.add,
                            )
                        else:
                            nc.scalar.activation(
                                out=tk[:, sl],
                                in_=pb[:, psl],
                                func=mybir.ActivationFunctionType.Identity,
                                bias=m_base_f[:, b:b + 1],
                            )
                        alt ^= 1
                    for t_sb, name in ((ti, "out_i"), (tj, "out_j"), (tk, "out_k")):
                        nc.sync.dma_start(
                            out=bass.AP(
                                tensor=out_d[name], offset=toff, ap=[[F, P], [1, F]]
                            ),
                            in_=t_sb[:, :F],
                        )

    nc.compile()
    return nc


def _gather(plan, results):
    perm = plan["perm"]
    outs = []
    for name in ("out_i", "out_j", "out_k"):
        scratch = np.concatenate(
            [results[k][name].reshape(-1) for k in range(plan["n_cores"])]
        )
        outs.append(np.ascontiguousarray(scratch[perm], dtype=np.int32))
    return tuple(outs)


def _enable_axon_tracing():
    """Register the ctypes NTFF hook (image's antenv lacks axon_hooks) and
    neuter the artifact upload (no bucket access in this container)."""
    import sys
    import types

    try:
        import antenv.axon_hooks as ah
    except ModuleNotFoundError:
        import antenv

        ah = types.ModuleType("antenv.axon_hooks")
        ah._HOOK = None
        ah.set_axon_ntff_profile_hook = lambda h: setattr(ah, "_HOOK", h)
        ah.get_axon_ntff_profile_hook = lambda: ah._HOOK
        sys.modules["antenv.axon_hooks"] = ah
        antenv.axon_hooks = ah

    if ah.get_axon_ntff_profile_hook() is None:
        from trn_agent_boot.trn_boot import _ntff_profile_via_ctypes

        ah.set_axon_ntff_profile_hook(
            _ntff_profile_via_ctypes("/opt/axon/libaxon_pjrt.so")
        )
    import concourse.bass_utils as bu

    bu.upload_artifacts = lambda tmpdir: str(tmpdir)


def run(idx_i, trace=False):
    from concourse.bass_utils import run_bass_kernel_spmd

    if trace:
        _enable_axon_tracing()
    plan = _plan(idx_i, N_CORES)
    nc = _build_program(plan)
    res = run_bass_kernel_spmd(
        nc,
        plan["in_maps"],
        list(range(N_CORES)),
        trace=trace,
        trace_cores=list(range(N_CORES)) if trace else None,
    )
    return _gather(plan, res.results), res


def kernel(idx_i):
    outs, _ = run(idx_i, trace=False)
    return outs



# revision 9
# speedup vs baseline: 1.4514x; 1.4514x over previous
"""CollectAtomTriples Trainium2 kernel.

Input: idx_i -- sorted int32 center indices [N_PAIRS] forming ragged segments.
Output: (idx_i_triples, idx_j_triples, idx_k_triples) -- for every segment of
length c, all C(c,2) unordered neighbor pairs (a<b, lexicographic), emitting
(segment_id, seg_start+a, seg_start+b) at data-dependent total length T.

Strategy (v4): host groups segments by count-class c and round-robins each
class's segments across the 8 cores (per-core class histograms equal +-1), so
all cores share one SPMD layout with near-zero padding: classes are walked in
descending pattern width M=C(c,2) and their per-core rows are stacked into
128-row "stacks" (stack width = widest class in it); stacks are greedy-packed
into [128, F<=F_MAX] tiles, each tile = 3 large contiguous HWDGE output DMAs.
Total scratch is ~1.04x the ideal 3*T/8 words vs 2.0x for the v3 class-block
layout -- output HBM writes (the roofline term) are nearly halved.

The v3 pattern-broadcast DMAs (40% of all SDMA engine time) are eliminated:
patterns live in DRAM as one bf16 row (values < 58, bf16-exact), and the idle
Tensor engine broadcasts each 1024-col phase window to all 128 partitions via
rank-1 matmuls (ones[1,128]^T @ pat[1,512]) into PSUM. The adds read the
PSUM operand directly: out_j on DVE (tensor_tensor psum + per-partition base
broadcast), out_k on ACT (activation identity, psum in, base bias), out_i on
GpSimd (tensor_copy of the segid column broadcast along free dim). DVE only
touches the a-window banks and ACT the b-window banks, so the engines read
PSUM in parallel. Output DMAs go on the sync (SP) HWDGE ring; small input
loads on the scalar (ACT) ring so phase rhs windows never queue behind 2MB
output writes. The host applies the static scratch->output permutation during
gather/unshard.
"""

import numpy as np

N_CORES = 8
P = 128
F_MAX = 4096  # output tile free-dim elems (16KB int32 per partition)
PHI = 1024    # pattern phase window; psum tile [128, 2*PHI] f32 = 4 banks


def _plan(idx, n_cores):
    idx = np.asarray(idx)
    n = idx.shape[0]
    starts = np.concatenate(
        [[0], np.flatnonzero(idx[1:] != idx[:-1]) + 1]
    ).astype(np.int64)
    counts = np.diff(np.concatenate([starts, [n]]))
    tri_counts = counts * (counts - 1) // 2
    ctri = np.cumsum(tri_counts)
    T = int(ctri[-1])
    tri_off = ctri - tri_counts  # exclusive scan

    sel = np.flatnonzero(tri_counts > 0)  # segments with c >= 2
    sc = counts[sel].astype(np.int64)
    soff = starts[sel]
    stri = tri_off[sel]

    # classes in DESCENDING count order (=> descending pattern width M)
    classes_asc, inv_asc = np.unique(sc, return_inverse=True)
    nC = classes_asc.size
    classes = classes_asc[::-1].copy()
    cidx = (nC - 1) - inv_asc  # class index (desc order) per segment
    N_c = np.bincount(cidx, minlength=nC)
    M_of = (classes * (classes - 1) // 2).astype(np.int64)
    assert int(M_of.max()) <= F_MAX

    # per-class segment lists, ascending global segment order
    order = np.argsort(cidx, kind="stable")
    class_ptr = np.concatenate([[0], np.cumsum(N_c)])

    # pattern tables in class order
    pat_off = np.concatenate([[0], np.cumsum(M_of)])
    L = int(pat_off[-1])
    pa = np.empty(L, np.int64)
    pb = np.empty(L, np.int64)
    for ci in range(nC):
        a, b = np.triu_indices(int(classes[ci]), 1)
        pa[pat_off[ci]:pat_off[ci + 1]] = a
        pb[pat_off[ci]:pat_off[ci + 1]] = b

    # phases: PHI-wide windows of the pattern table; patw row interleaves
    # [a-window | b-window] per phase so one DMA feeds each phase's matmuls
    n_ph = -(-L // PHI)
    w_p = [min(PHI, L - p * PHI) for p in range(n_ph)]
    import ml_dtypes

    patw = np.empty(2 * L, ml_dtypes.bfloat16)
    for p in range(n_ph):
        o, w = p * PHI, w_p[p]
        patw[2 * o:2 * o + w] = pa[o:o + w]
        patw[2 * o + w:2 * o + 2 * w] = pb[o:o + w]
    assert int(pa.max()) < 256 and int(pb.max()) < 256  # bf16-exact

    # stacks (128-row columns) + placements (class row-runs within stacks)
    H = -(-N_c // n_cores)  # per-core capacity rows per class
    stacks = []  # dict: w, rows, tile, col0
    placements = []  # dict: ci, st, r0, nr, rank0, chunks
    for ci in range(nC):
        rows = int(H[ci])
        rank0 = 0
        first = True
        while rows > 0:
            if stacks and first:
                # compute ops must start at a partition multiple of 32:
                # round this class's start row up within the current stack
                stacks[-1]["rows"] = -(-stacks[-1]["rows"] // 32) * 32
            first = False
            if not stacks or stacks[-1]["rows"] >= P:
                stacks.append({"w": int(M_of[ci]), "rows": 0})
            st = stacks[-1]
            take = min(rows, P - st["rows"])
            placements.append(
                {"ci": ci, "st": len(stacks) - 1, "r0": st["rows"],
                 "nr": take, "rank0": rank0}
            )
            st["rows"] += take
            rows -= take
            rank0 += take

    # greedy-pack stacks into [128, F<=F_MAX] tiles
    tiles = []  # (dram_off, F)
    cur_w, cur_off = 0, 0
    for st in stacks:
        if cur_w + st["w"] > F_MAX and cur_w > 0:
            tiles.append((cur_off, cur_w))
            cur_off += P * cur_w
            cur_w = 0
        st["tile"] = len(tiles)
        st["col0"] = cur_w
        cur_w += st["w"]
    tiles.append((cur_off, cur_w))
    S_total = cur_off + P * cur_w

    # pattern chunks per placement: (phase, clo, chi, acol)
    for pl in placements:
        o = int(pat_off[pl["ci"]])
        M = int(M_of[pl["ci"]])
        chunks = []
        for p in range(o // PHI, (o + M - 1) // PHI + 1):
            lo = max(o, p * PHI)
            hi = min(o + M, p * PHI + w_p[p])
            chunks.append((p, lo - o, hi - o, lo - p * PHI))
        pl["chunks"] = chunks

    # per-core metadata + host-side gather permutation
    NP = len(placements)
    nS = len(stacks)
    m_segid = np.zeros((n_cores, P, nS), np.int32)  # per-STACK segid columns
    m_base_f = np.zeros((n_cores, P, NP), np.float32)
    perm = np.empty(T, np.int64)
    rot_of = np.concatenate([[0], np.cumsum(N_c)]) % n_cores
    for pcol, pl in enumerate(placements):
        ci = pl["ci"]
        M = int(M_of[ci])
        g = order[class_ptr[ci]:class_ptr[ci + 1]]
        st = stacks[pl["st"]]
        toff, F = tiles[st["tile"]]
        addr0 = toff + st["col0"]
        rotc = int(rot_of[ci])
        js = np.arange(pl["rank0"], pl["rank0"] + pl["nr"])
        marange = np.arange(M)
        for k in range(n_cores):
            gr = ((k - rotc) % n_cores) + n_cores * js  # global ranks
            valid = gr < N_c[ci]
            if not valid.any():
                continue
            jv = js[valid]
            ss = g[gr[valid]]
            rows = pl["r0"] + (jv - pl["rank0"])
            m_segid[k, rows, pl["st"]] = sel[ss].astype(np.int32)
            m_base_f[k, rows, pcol] = soff[ss].astype(np.float32)
            addr = k * S_total + addr0 + rows * F
            perm[(stri[ss][:, None] + marange[None, :]).ravel()] = (
                addr[:, None] + marange[None, :]
            ).ravel()

    import ml_dtypes

    ones = np.ones((1, P), ml_dtypes.bfloat16)
    in_maps = [
        {
            "m_segid": m_segid[k],
            "m_base_f": m_base_f[k],
            "patw": patw[None, :],
            "ones": ones,
        }
        for k in range(n_cores)
    ]
    return {
        "NP": NP,
        "nS": nS,
        "L": L,
        "n_ph": n_ph,
        "w_p": w_p,
        "M_of": M_of,
        "placements": placements,
        "stacks": stacks,
        "tiles": tiles,
        "T": T,
        "S_total": S_total,
        "perm": perm,
        "in_maps": in_maps,
        "n_cores": n_cores,
    }


def _build_program(plan):
    import concourse.bacc as bacc
    import concourse.bass as bass
    import concourse.mybir as mybir
    import concourse.tile as tile

    NP = plan["NP"]
    nS = plan["nS"]
    L = plan["L"]
    w_p = plan["w_p"]
    S_total = plan["S_total"]
    i32 = mybir.dt.int32
    f32 = mybir.dt.float32
    bf16 = mybir.dt.bfloat16

    nc = bacc.Bacc(
        "TRN2",
        target_bir_lowering=False,
        debug=False,
        num_devices=plan["n_cores"],
    )
    m_segid_d = nc.dram_tensor("m_segid", [P, nS], i32, kind="ExternalInput")
    m_base_f_d = nc.dram_tensor("m_base_f", [P, NP], f32, kind="ExternalInput")
    patw_d = nc.dram_tensor("patw", [1, 2 * L], bf16, kind="ExternalInput")
    ones_d = nc.dram_tensor("ones", [1, P], bf16, kind="ExternalInput")
    out_d = {
        name: nc.dram_tensor(name, [S_total, 1], i32, kind="ExternalOutput")
        for name in ("out_i", "out_j", "out_k")
    }

    with tile.TileContext(nc) as tc:
        with (
            tc.tile_pool(name="meta", bufs=1) as meta_pool,
            tc.tile_pool(name="rhs", bufs=3) as rhs_pool,
            tc.tile_pool(name="psum", bufs=2, space="PSUM") as psum_pool,
            tc.tile_pool(name="work", bufs=3) as work_pool,
        ):
            m_segid = meta_pool.tile([P, nS], i32, tag="msegid")
            m_base_f = meta_pool.tile([P, NP], f32, tag="mbasef")
            ones = meta_pool.tile([1, P], bf16, tag="ones")
            nc.scalar.dma_start(out=m_segid[:], in_=m_segid_d.ap())
            nc.scalar.dma_start(out=m_base_f[:], in_=m_base_f_d.ap())
            nc.scalar.dma_start(out=ones[:], in_=ones_d.ap())

            pps = {}

            def emit_phase(p):
                w = w_p[p]
                rhs = rhs_pool.tile([1, 2 * PHI], bf16, tag="rhs")
                nc.scalar.dma_start(
                    out=rhs[0:1, 0:2 * w],
                    in_=bass.AP(
                        tensor=patw_d, offset=2 * p * PHI, ap=[[0, 1], [1, 2 * w]]
                    ),
                )
                pp = psum_pool.tile([P, 2 * PHI], f32, tag="pp")
                for k0 in range(0, 2 * w, 512):
                    kw = min(512, 2 * w - k0)
                    nc.tensor.matmul(
                        pp[:, k0:k0 + kw], ones[0:1, :], rhs[0:1, k0:k0 + kw]
                    )
                pps[p] = pp
                if p - 2 in pps:
                    del pps[p - 2]

            emitted_ph = -1
            cur_tile = -1
            ti = tj = tk = None

            def flush_tile():
                toff, F = plan["tiles"][cur_tile]
                for t_sb, name in ((ti, "out_i"), (tj, "out_j"), (tk, "out_k")):
                    nc.sync.dma_start(
                        out=bass.AP(
                            tensor=out_d[name], offset=toff, ap=[[F, P], [1, F]]
                        ),
                        in_=t_sb[:, :F],
                    )

            for pcol, pl in enumerate(plan["placements"]):
                st = plan["stacks"][pl["st"]]
                if st["tile"] != cur_tile:
                    if cur_tile >= 0:
                        flush_tile()
                    cur_tile = st["tile"]
                    ti = work_pool.tile([P, F_MAX], i32, tag="ti")
                    tj = work_pool.tile([P, F_MAX], i32, tag="tj")
                    tk = work_pool.tile([P, F_MAX], i32, tag="tk")
                    # stream i: one full-partition GpSimd copy per stack of
                    # this tile (Pool ops cannot start at a partition offset)
                    for si in range(len(plan["stacks"])):
                        sst = plan["stacks"][si]
                        if sst["tile"] != cur_tile:
                            continue
                        nc.gpsimd.tensor_copy(
                            out=ti[:, sst["col0"]:sst["col0"] + sst["w"]],
                            in_=m_segid[:, si:si + 1].to_broadcast(
                                [P, sst["w"]]
                            ),
                        )
                col0 = st["col0"]
                # engines address partitions as aligned power-of-2 blocks:
                # start 0 reaches 128 rows, start 64 reaches 64, 32/96 reach 32
                blocks = []
                br, bn = pl["r0"], pl["nr"]
                while bn > 0:
                    reach = {0: 128, 32: 32, 64: 64, 96: 32}[br]
                    take = min(bn, reach)
                    blocks.append((br, take))
                    br += take
                    bn -= take
                for (p, clo, chi, acol) in pl["chunks"]:
                    while emitted_ph < p:
                        emitted_ph += 1
                        emit_phase(emitted_ph)
                    w = w_p[p]
                    wch = chi - clo
                    for (br, bn) in blocks:
                        rows = slice(br, br + bn)
                        nc.vector.tensor_tensor(
                            out=tj[rows, col0 + clo:col0 + chi],
                            in0=pps[p][rows, acol:acol + wch],
                            in1=m_base_f[rows, pcol:pcol + 1].to_broadcast(
                                [bn, wch]
                            ),
                            op=mybir.AluOpType.add,
                        )
                        nc.scalar.activation(
                            out=tk[rows, col0 + clo:col0 + chi],
                            in_=pps[p][rows, w + acol:w + acol + wch],
                            func=mybir.ActivationFunctionType.Identity,
                            bias=m_base_f[rows, pcol:pcol + 1],
                        )
            flush_tile()

    nc.compile()
    return nc


def _gather(plan, results):
    perm = plan["perm"]
    outs = []
    for name in ("out_i", "out_j", "out_k"):
        scratch = np.concatenate(
            [results[k][name].reshape(-1) for k in range(plan["n_cores"])]
        )
        outs.append(np.ascontiguousarray(scratch[perm], dtype=np.int32))
    return tuple(outs)


def _enable_axon_tracing():
    """Register the ctypes NTFF hook (image's antenv lacks axon_hooks) and
    neuter the artifact upload (no bucket access in this container)."""
    import sys
    import types

    try:
        import antenv.axon_hooks as ah
    except ModuleNotFoundError:
        import antenv

        ah = types.ModuleType("antenv.axon_hooks")
        ah._HOOK = None
        ah.set_axon_ntff_profile_hook = lambda h: setattr(ah, "_HOOK", h)
        ah.get_axon_ntff_profile_hook = lambda: ah._HOOK
        sys.modules["antenv.axon_hooks"] = ah
        antenv.axon_hooks = ah

    if ah.get_axon_ntff_profile_hook() is None:
        from trn_agent_boot.trn_boot import _ntff_profile_via_ctypes

        ah.set_axon_ntff_profile_hook(
            _ntff_profile_via_ctypes("/opt/axon/libaxon_pjrt.so")
        )
    import concourse.bass_utils as bu

    bu.upload_artifacts = lambda tmpdir: str(tmpdir)


def run(idx_i, trace=False):
    from concourse.bass_utils import run_bass_kernel_spmd

    if trace:
        _enable_axon_tracing()
    plan = _plan(idx_i, N_CORES)
    nc = _build_program(plan)
    res = run_bass_kernel_spmd(
        nc,
        plan["in_maps"],
        list(range(N_CORES)),
        trace=trace,
        trace_cores=list(range(N_CORES)) if trace else None,
    )
    return _gather(plan, res.results), res


def kernel(idx_i):
    outs, _ = run(idx_i, trace=False)
    return outs
